# revision 25
# baseline (speedup 1.0000x reference)
"""AffinityEnergyLoss on 8 Trainium2 NeuronCores (Bass/Tile).

Sharding: core k handles (layer l = k // 4, batch b = k % 4) — one
(l, b) slab of the encoder attns (8 heads x 1025 x 1025, CLS row/col
cropped) plus the matching slab of decoder attns (8 x 1024 x 1024),
~67 MB per core. The kernel is HBM-bandwidth bound.

Per core, for each of its 16 maps M (1024 x 1024 fp32), in 128-row
blocks streamed via SWDGE cast-DMA (fp32 -> fp32r in flight):
    rowsum s = M @ 1     (DVE reduce_sum / ACT activation accum_out,
                          alternating per map to split the load)
    r = 1/s              (DVE reciprocal)
    S += diag(r) @ M     (PE fp32r matmul at 1 cyc/row, accumulated in
                          PSUM over all 16 maps; diag(r) built as eye*r)
so S = sum_m D_m M_m. Then per block:
    T = S^T              (PE transpose via identity)
    Z^T = Pa^T @ T       (fp32r PE matmul; Pa = softmax(preds_b) built
                          on-device from host-transposed preds)
Tiny bf16 keep-warm matmuls are interleaved so the PE HAM clock gate
stays at K=8/8; otherwise cold-clock matmuls cap the pipeline below
DMA pace. Z (1024 x 21) is the core's partial of sum_m D_m M_m @ P.

Host: affinity_raw_b = (Z_{l=0,b} + Z_{l=1,b}) / 32, row-normalize,
then loss = sum(roi * |softmax(preds) - affinity|) / N  (the per-batch
combine + scalar reduction the sharding hint calls the "all-reduce").

Measured: ~196-220 us HW exec (8 cores), rel err ~6e-7 vs the fp32
reference (fp32r mantissa truncation averages out across 32 maps).
"""
import numpy as np

import concourse.bacc as bacc
import concourse.mybir as mybir
import concourse.tile as tile
from concourse.bass_utils import run_bass_kernel_spmd

F32 = mybir.dt.float32
F32R = mybir.dt.float32r
AX = mybir.AxisListType.X
ACTF = mybir.ActivationFunctionType

HEADS = 8
TOK = 1024
C = 21
PB = 128          # partition block
NBLK = TOK // PB  # 8

_NC = None


def _build_nc():
    nc = bacc.Bacc(None, target_bir_lowering=False)
    enc = nc.dram_tensor("enc", [HEADS, 1025, 1025], F32, kind="ExternalInput")
    dec = nc.dram_tensor("dec", [HEADS, TOK, TOK], F32, kind="ExternalInput")
    pt = nc.dram_tensor("pt", [TOK, C], F32, kind="ExternalInput")
    eye = nc.dram_tensor("eye", [PB, PB], F32, kind="ExternalInput")
    z = nc.dram_tensor("z", [NBLK, C, PB], F32, kind="ExternalOutput")

    def _head_src(enc, dec, ib, m, nh):
        # heads m..m+nh of the block's 16 maps (0-7 enc, 8-15 dec)
        i0 = ib * PB
        if m < 8:
            return enc[m : m + nh, 1 + i0 : 1 + i0 + PB, 1:].transpose([1, 0, 2])
        return dec[m - 8 : m - 8 + nh, i0 : i0 + PB, :].transpose([1, 0, 2])

    def _chunk_plan(ib):
        # (start_map, n_heads) per DMA chunk for block ib
        if ib == 0:
            # small leading chunks so the first bytes land ASAP
            return [(0, 1), (1, 1), (2, 1), (3, 1), (4, 4), (8, 4), (12, 4)]
        if ib == NBLK - 1:
            # small trailing chunks to shrink the serial tail
            return [(0, 4), (4, 4), (8, 4), (12, 2), (14, 1), (15, 1)]
        return [(0, 4), (4, 4), (8, 4), (12, 4)]

    with tile.TileContext(nc) as tc:
        with (
            tc.tile_pool(name="const", bufs=1) as const,
            tc.tile_pool(name="stats", bufs=8) as stats,
            tc.tile_pool(name="big", bufs=8) as big,
            tc.tile_pool(name="spool", bufs=2) as spool,
            tc.tile_pool(name="zout", bufs=2) as zout,
            tc.tile_pool(name="psS", bufs=2, space="PSUM") as psS,
            tc.tile_pool(name="psT", bufs=2, space="PSUM") as psT,
            tc.tile_pool(name="psZ", bufs=1, space="PSUM") as psZ,
            tc.tile_pool(name="psW", bufs=1, space="PSUM") as psW,
        ):
            # issue the first block's big loads before anything else.
            # The first two 1-head chunks go on the HWDGE (sync) rail: its
            # RTL descriptor generation starts right after the entry
            # barrier, ~2 us before the Q7 SWDGE path emits its first
            # descriptors. ACT rounds those two maps to f32r during the
            # rowsum pass.
            chunk_tiles = {}
            for ci, (m0, nh) in enumerate(_chunk_plan(0)):
                if ci < 4:
                    t = big.tile(
                        [PB, 1, TOK], F32, tag="chunkf", name=f"pref{ci}", bufs=4
                    )
                    nc.sync.dma_start(
                        out=t[:, 0:nh, :], in_=_head_src(enc, dec, 0, m0, nh)
                    )
                else:
                    t = big.tile([PB, 4, TOK], F32R, tag="chunk", name=f"pre{ci}")
                    nc.gpsimd.dma_start(
                        out=t[:, 0:nh, :], in_=_head_src(enc, dec, 0, m0, nh)
                    )
                chunk_tiles[(0, ci)] = t

            eye_sb = const.tile([PB, PB], F32)
            nc.sync.dma_start(out=eye_sb[:], in_=eye[:])

            # keep-warm: tiny bf16 matmuls interleaved with the real stream
            # keep the PE HAM activity monitor busy enough to hold K=8/8.
            wu_a = const.tile([PB, 64], mybir.dt.bfloat16)
            nc.vector.memset(wu_a[:], 0.0)
            wu_ps = psW.tile([PB, 64], F32)
            wu_n = [0]

            def _warm(k=1):
                for _ in range(k):
                    nc.tensor.matmul(
                        wu_ps[0:64, :], wu_a[:, 0:64], wu_a[:],
                        start=(wu_n[0] == 0), stop=False,
                    )
                    wu_n[0] += 1

            pt_sb = const.tile([PB, NBLK, C], F32)
            nc.sync.dma_start(
                out=pt_sb[:], in_=pt.rearrange("(c p) n -> p c n", p=PB)
            )
            pa_sb = const.tile([PB, NBLK, C], F32R)
            for c in range(NBLK):
                negmx = stats.tile([PB, 1], F32, tag="negmx")
                nc.vector.reduce_max(negmx[:], pt_sb[:, c, :], axis=AX, negate=True)
                ssum = stats.tile([PB, 1], F32, tag="ssum")
                ex = stats.tile([PB, C], F32, tag="ex")
                nc.scalar.activation(
                    ex[:],
                    pt_sb[:, c, :],
                    ACTF.Exp,
                    bias=negmx[:],
                    accum_out=ssum[:],
                )
                rs = stats.tile([PB, 1], F32, tag="rs")
                nc.vector.reciprocal(rs[:], ssum[:])
                nc.vector.tensor_scalar_mul(pa_sb[:, c, :], ex[:], rs[:])

            for ib in range(NBLK):
                S_ps = psS.tile([PB, TOK], F32)
                for ci, (m0, nh) in enumerate(_chunk_plan(ib)):
                    t = chunk_tiles.pop((ib, ci), None)
                    if t is None:
                        t = big.tile([PB, 4, TOK], F32R, tag="chunk")
                        nc.gpsimd.dma_start(
                            out=t[:, 0:nh, :], in_=_head_src(enc, dec, ib, m0, nh)
                        )
                    for hm in range(nh):
                        m = m0 + hm
                        s_m = stats.tile([PB, 1], F32, tag="s_m")
                        if t.dtype == F32:
                            # HWDGE-loaded f32 chunk: ACT rounds to f32r
                            # while computing the rowsum
                            src = spool.tile([PB, TOK], F32R, tag="rnd", bufs=2)
                            nc.scalar.activation(
                                src[:], t[:, hm, :], ACTF.Copy, accum_out=s_m[:]
                            )
                        elif m % 2 == 0:
                            src = t[:, hm, :]
                            nc.vector.reduce_sum(s_m[:], src.bitcast(F32), axis=AX)
                        else:
                            src = t[:, hm, :]
                            scr = spool.tile([PB, TOK], F32, tag="scr")
                            nc.scalar.activation(
                                scr[:], src.bitcast(F32), ACTF.Copy, accum_out=s_m[:]
                            )
                        r_m = stats.tile([PB, 1], F32, tag="r_m")
                        nc.vector.reciprocal(r_m[:], s_m[:])
                        dg = stats.tile([PB, PB], F32R, tag="dg")
                        nc.vector.tensor_scalar_mul(dg[:], eye_sb[:], r_m[:])
                        nc.tensor.matmul(
                            S_ps[:, 0:512], dg[:], src[:, 0:512],
                            start=(m == 0), stop=(m == 15),
                        )
                        nc.tensor.matmul(
                            S_ps[:, 512:1024], dg[:], src[:, 512:1024],
                            start=(m == 0), stop=(m == 15),
                        )
                        _warm(2)

                S_sb = spool.tile([PB, TOK], F32, tag="S")
                nc.scalar.copy(out=S_sb[:], in_=S_ps[:])
                T_sb = spool.tile([PB, NBLK, PB], F32R, tag="T")
                for jc in range(NBLK):
                    tp = psT.tile([PB, PB], F32)
                    nc.tensor.transpose(
                        tp[:], S_sb[:, jc * PB : (jc + 1) * PB], eye_sb[:]
                    )
                    nc.scalar.copy(out=T_sb[:, jc, :], in_=tp[:])
                z_ps = psZ.tile([C, PB], F32)
                for jc in range(NBLK):
                    nc.tensor.matmul(
                        z_ps[:], pa_sb[:, jc, :], T_sb[:, jc, :],
                        start=(jc == 0), stop=(jc == NBLK - 1),
                    )
                z_sb = zout.tile([C, PB], F32)
                nc.vector.tensor_copy(z_sb[:], z_ps[:])
                nc.sync.dma_start(out=z[ib, :, :], in_=z_sb[:])

    nc.compile()
    return nc


def _get_nc():
    global _NC
    if _NC is None:
        _NC = _build_nc()
    return _NC


def kernel(preds, low_feats, high_feats, unlabeled_ROIs, targets, attns, decode_attns):
    preds = np.asarray(preds, dtype=np.float32)
    attns = np.asarray(attns, dtype=np.float32)
    decode_attns = np.asarray(decode_attns, dtype=np.float32)
    roi = np.asarray(unlabeled_ROIs)

    bz = preds.shape[0]
    preds_t = np.ascontiguousarray(
        preds.reshape(bz, C, TOK).transpose(0, 2, 1)
    )  # (bz, 1024, 21)
    eye_np = np.eye(PB, dtype=np.float32)

    nc = _get_nc()
    in_maps = []
    for k in range(8):
        l, b = k // 4, k % 4
        in_maps.append(
            {
                "enc": np.ascontiguousarray(attns[l, b]),
                "dec": np.ascontiguousarray(decode_attns[l, b]),
                "pt": preds_t[b],
                "eye": eye_np,
            }
        )
    res = run_bass_kernel_spmd(nc, in_maps, core_ids=list(range(8)))
    # z per core: (NBLK, C, PB) holding Z^T per block -> (1024, 21)
    zs = np.stack(
        [
            res.results[k]["z"].transpose(0, 2, 1).reshape(TOK, C)
            for k in range(8)
        ]
    )

    # combine: affinity_raw_b = (Z_{l=0,b} + Z_{l=1,b}) / 32
    zb = zs.reshape(2, bz, TOK, C).sum(axis=0) / 32.0
    aff = zb / zb.sum(axis=-1, keepdims=True)

    # host softmax (matches jax.nn.softmax in f32)
    e = np.exp(preds_t - preds_t.max(axis=-1, keepdims=True))
    prob = e / e.sum(axis=-1, keepdims=True)  # (bz, 1024, 21)

    roi_f = roi.astype(np.float32).reshape(bz, TOK, 1)
    n_roi = roi_f.sum()
    loss = (roi_f * np.abs(prob - aff)).sum()
    if n_roi > 0:
        loss = loss / n_roi
    return np.asarray(loss, dtype=np.float32)


# revision 26
# speedup vs baseline: 1.0957x; 1.0957x over previous
"""AffinityEnergyLoss on 8 Trainium2 NeuronCores (Bass/Tile).

Sharding: core k handles (layer l = k // 4, batch b = k % 4) — one
(l, b) slab of the encoder attns (8 heads x 1025 x 1025, CLS row/col
cropped) plus the matching slab of decoder attns (8 x 1024 x 1024),
~67 MB per core. The kernel is HBM-bandwidth bound.

Per core, for each of its 16 maps M (1024 x 1024 fp32), in 128-row
blocks streamed via SWDGE cast-DMA (fp32 -> fp32r in flight):
    rowsum s = M @ 1     (DVE reduce_sum / ACT activation accum_out,
                          alternating per map to split the load)
    r = 1/s              (DVE reciprocal)
    S += diag(r) @ M     (PE fp32r matmul at 1 cyc/row, accumulated in
                          PSUM over all 16 maps; diag(r) built as eye*r)
so S = sum_m D_m M_m. Then per block:
    T = S^T              (PE transpose via identity)
    Z^T = Pa^T @ T       (fp32r PE matmul; Pa = softmax(preds_b) built
                          on-device from host-transposed preds)
Tiny bf16 keep-warm matmuls are interleaved so the PE HAM clock gate
stays at K=8/8; otherwise cold-clock matmuls cap the pipeline below
DMA pace. Z (1024 x 21) is the core's partial of sum_m D_m M_m @ P.

Host: affinity_raw_b = (Z_{l=0,b} + Z_{l=1,b}) / 32, row-normalize,
then loss = sum(roi * |softmax(preds) - affinity|) / N  (the per-batch
combine + scalar reduction the sharding hint calls the "all-reduce").

Measured: ~196-220 us HW exec (8 cores), rel err ~6e-7 vs the fp32
reference (fp32r mantissa truncation averages out across 32 maps).
"""
import numpy as np

import concourse.bacc as bacc
import concourse.mybir as mybir
import concourse.tile as tile
from concourse.bass_utils import run_bass_kernel_spmd

F32 = mybir.dt.float32
F32R = mybir.dt.float32r
AX = mybir.AxisListType.X
ACTF = mybir.ActivationFunctionType

HEADS = 8
TOK = 1024
C = 21
PB = 128          # partition block
NBLK = TOK // PB  # 8

_NC = None


def _build_nc():
    nc = bacc.Bacc(None, target_bir_lowering=False)
    enc = nc.dram_tensor("enc", [HEADS, 1025, 1025], F32, kind="ExternalInput")
    dec = nc.dram_tensor("dec", [HEADS, TOK, TOK], F32, kind="ExternalInput")
    pt = nc.dram_tensor("pt", [TOK, C], F32, kind="ExternalInput")
    eye = nc.dram_tensor("eye", [PB, PB], F32, kind="ExternalInput")
    z = nc.dram_tensor("z", [NBLK, C, PB], F32, kind="ExternalOutput")

    def _head_src(enc, dec, ib, m, nh):
        # heads m..m+nh of the block's 16 maps (0-7 enc, 8-15 dec)
        i0 = ib * PB
        if m < 8:
            return enc[m : m + nh, 1 + i0 : 1 + i0 + PB, 1:].transpose([1, 0, 2])
        return dec[m - 8 : m - 8 + nh, i0 : i0 + PB, :].transpose([1, 0, 2])

    def _chunk_plan(ib):
        # (start_map, n_heads) per DMA chunk for block ib
        if ib == 0:
            # small leading chunks so the first bytes land ASAP
            return [(0, 1), (1, 1), (2, 2), (4, 4), (8, 4), (12, 4)]
        if ib == NBLK - 1:
            # small trailing chunks to shrink the serial tail
            return [(0, 4), (4, 4), (8, 4), (12, 2), (14, 1), (15, 1)]
        return [(0, 4), (4, 4), (8, 4), (12, 4)]

    with tile.TileContext(nc) as tc:
        with (
            tc.tile_pool(name="const", bufs=1) as const,
            tc.tile_pool(name="stats", bufs=8) as stats,
            tc.tile_pool(name="big", bufs=8) as big,
            tc.tile_pool(name="spool", bufs=2) as spool,
            tc.tile_pool(name="zout", bufs=2) as zout,
            tc.tile_pool(name="psS", bufs=2, space="PSUM") as psS,
            tc.tile_pool(name="psT", bufs=2, space="PSUM") as psT,
            tc.tile_pool(name="psZ", bufs=1, space="PSUM") as psZ,
            tc.tile_pool(name="psW", bufs=1, space="PSUM") as psW,
        ):
            # issue the first block's big loads before anything else.
            # The first two 1-head chunks go on the HWDGE (sync) rail: its
            # RTL descriptor generation starts right after the entry
            # barrier, ~2 us before the Q7 SWDGE path emits its first
            # descriptors. ACT rounds those two maps to f32r during the
            # rowsum pass.
            chunk_tiles = {}
            for ci, (m0, nh) in enumerate(_chunk_plan(0)):
                if ci < 2:
                    t = big.tile(
                        [PB, 1, TOK], F32, tag="chunkf", name=f"pref{ci}", bufs=2
                    )
                    nc.sync.dma_start(
                        out=t[:, 0:nh, :], in_=_head_src(enc, dec, 0, m0, nh)
                    )
                else:
                    t = big.tile([PB, 4, TOK], F32R, tag="chunk", name=f"pre{ci}")
                    nc.gpsimd.dma_start(
                        out=t[:, 0:nh, :], in_=_head_src(enc, dec, 0, m0, nh)
                    )
                chunk_tiles[(0, ci)] = t

            eye_sb = const.tile([PB, PB], F32)
            nc.sync.dma_start(out=eye_sb[:], in_=eye[:])

            # keep-warm: tiny bf16 matmuls interleaved with the real stream
            # keep the PE HAM activity monitor busy enough to hold K=8/8.
            wu_a = const.tile([PB, 64], mybir.dt.bfloat16)
            nc.vector.memset(wu_a[:], 0.0)
            wu_ps = psW.tile([PB, 64], F32)
            wu_n = [0]

            def _warm(k=1):
                for _ in range(k):
                    nc.tensor.matmul(
                        wu_ps[0:64, :], wu_a[:, 0:64], wu_a[:],
                        start=(wu_n[0] == 0), stop=False,
                    )
                    wu_n[0] += 1

            pt_sb = const.tile([PB, NBLK, C], F32)
            nc.sync.dma_start(
                out=pt_sb[:], in_=pt.rearrange("(c p) n -> p c n", p=PB)
            )
            pa_sb = const.tile([PB, NBLK, C], F32R)
            for c in range(NBLK):
                negmx = stats.tile([PB, 1], F32, tag="negmx")
                nc.vector.reduce_max(negmx[:], pt_sb[:, c, :], axis=AX, negate=True)
                ssum = stats.tile([PB, 1], F32, tag="ssum")
                ex = stats.tile([PB, C], F32, tag="ex")
                nc.scalar.activation(
                    ex[:],
                    pt_sb[:, c, :],
                    ACTF.Exp,
                    bias=negmx[:],
                    accum_out=ssum[:],
                )
                rs = stats.tile([PB, 1], F32, tag="rs")
                nc.vector.reciprocal(rs[:], ssum[:])
                nc.vector.tensor_scalar_mul(pa_sb[:, c, :], ex[:], rs[:])

            for ib in range(NBLK):
                S_ps = psS.tile([PB, TOK], F32)
                for ci, (m0, nh) in enumerate(_chunk_plan(ib)):
                    t = chunk_tiles.pop((ib, ci), None)
                    if t is None:
                        t = big.tile([PB, 4, TOK], F32R, tag="chunk")
                        nc.gpsimd.dma_start(
                            out=t[:, 0:nh, :], in_=_head_src(enc, dec, ib, m0, nh)
                        )
                    for hm in range(nh):
                        m = m0 + hm
                        s_m = stats.tile([PB, 1], F32, tag="s_m")
                        if t.dtype == F32:
                            # HWDGE-loaded f32 chunk: ACT rounds to f32r
                            # while computing the rowsum
                            src = spool.tile([PB, TOK], F32R, tag="rnd", bufs=2)
                            nc.scalar.activation(
                                src[:], t[:, hm, :], ACTF.Copy, accum_out=s_m[:]
                            )
                        elif m % 2 == 0:
                            src = t[:, hm, :]
                            nc.vector.reduce_sum(s_m[:], src.bitcast(F32), axis=AX)
                        else:
                            src = t[:, hm, :]
                            scr = spool.tile([PB, TOK], F32, tag="scr")
                            nc.scalar.activation(
                                scr[:], src.bitcast(F32), ACTF.Copy, accum_out=s_m[:]
                            )
                        r_m = stats.tile([PB, 1], F32, tag="r_m")
                        nc.vector.reciprocal(r_m[:], s_m[:])
                        dg = stats.tile([PB, PB], F32R, tag="dg")
                        nc.vector.tensor_scalar_mul(dg[:], eye_sb[:], r_m[:])
                        nc.tensor.matmul(
                            S_ps[:, 0:512], dg[:], src[:, 0:512],
                            start=(m == 0), stop=(m == 15),
                        )
                        nc.tensor.matmul(
                            S_ps[:, 512:1024], dg[:], src[:, 512:1024],
                            start=(m == 0), stop=(m == 15),
                        )
                        _warm(2)

                S_sb = spool.tile([PB, TOK], F32, tag="S")
                nc.scalar.copy(out=S_sb[:], in_=S_ps[:])
                T_sb = spool.tile([PB, NBLK, PB], F32R, tag="T")
                for jc in range(NBLK):
                    tp = psT.tile([PB, PB], F32)
                    nc.tensor.transpose(
                        tp[:], S_sb[:, jc * PB : (jc + 1) * PB], eye_sb[:]
                    )
                    nc.scalar.copy(out=T_sb[:, jc, :], in_=tp[:])
                z_ps = psZ.tile([C, PB], F32)
                for jc in range(NBLK):
                    nc.tensor.matmul(
                        z_ps[:], pa_sb[:, jc, :], T_sb[:, jc, :],
                        start=(jc == 0), stop=(jc == NBLK - 1),
                    )
                z_sb = zout.tile([C, PB], F32)
                nc.vector.tensor_copy(z_sb[:], z_ps[:])
                nc.sync.dma_start(out=z[ib, :, :], in_=z_sb[:])

    nc.compile()
    return nc


def _get_nc():
    global _NC
    if _NC is None:
        _NC = _build_nc()
    return _NC


def kernel(preds, low_feats, high_feats, unlabeled_ROIs, targets, attns, decode_attns):
    preds = np.asarray(preds, dtype=np.float32)
    attns = np.asarray(attns, dtype=np.float32)
    decode_attns = np.asarray(decode_attns, dtype=np.float32)
    roi = np.asarray(unlabeled_ROIs)

    bz = preds.shape[0]
    preds_t = np.ascontiguousarray(
        preds.reshape(bz, C, TOK).transpose(0, 2, 1)
    )  # (bz, 1024, 21)
    eye_np = np.eye(PB, dtype=np.float32)

    nc = _get_nc()
    in_maps = []
    for k in range(8):
        l, b = k // 4, k % 4
        in_maps.append(
            {
                "enc": np.ascontiguousarray(attns[l, b]),
                "dec": np.ascontiguousarray(decode_attns[l, b]),
                "pt": preds_t[b],
                "eye": eye_np,
            }
        )
    res = run_bass_kernel_spmd(nc, in_maps, core_ids=list(range(8)))
    # z per core: (NBLK, C, PB) holding Z^T per block -> (1024, 21)
    zs = np.stack(
        [
            res.results[k]["z"].transpose(0, 2, 1).reshape(TOK, C)
            for k in range(8)
        ]
    )

    # combine: affinity_raw_b = (Z_{l=0,b} + Z_{l=1,b}) / 32
    zb = zs.reshape(2, bz, TOK, C).sum(axis=0) / 32.0
    aff = zb / zb.sum(axis=-1, keepdims=True)

    # host softmax (matches jax.nn.softmax in f32)
    e = np.exp(preds_t - preds_t.max(axis=-1, keepdims=True))
    prob = e / e.sum(axis=-1, keepdims=True)  # (bz, 1024, 21)

    roi_f = roi.astype(np.float32).reshape(bz, TOK, 1)
    n_roi = roi_f.sum()
    loss = (roi_f * np.abs(prob - aff)).sum()
    if n_roi > 0:
        loss = loss / n_roi
    return np.asarray(loss, dtype=np.float32)
